# revision 1
# baseline (speedup 1.0000x reference)
"""BlockReLU Trainium2 kernel.

Full input: activation [32, 128, 112, 112] f32. Channel groups:
  [0,64): 1x1 blocks (plain ReLU), [64,96): 2x2 blocks, [96,120): 4x4 blocks,
  [120,128): identity passthrough.
A block's mask is 1 where the block's spatial sum >= 0, else 0; the mask is
broadcast over the block and multiplies the input.

Strategy: pure data parallelism over batch N across 8 NeuronCores (4 images
per core). Per core, stream H in chunks of 28 rows. For each chunk, pack
(channel, image) pairs onto all 128 SBUF partitions so every engine op uses
every lane:
  G1 relu:  two tiles, p = c*2 + n_local      (64ch x 2img) -> ScalarE Relu
  G2 2x2:   one tile,  p = (c-64)*4 + n       (32ch x 4img) -> VectorE
  G3+G4:    one tile,  p = (c-96)*4 + n       (32ch x 4img) -> VectorE
            (partitions 0:96 are the 4x4-mask channels, 96:128 identity)
Block sums are strided pairwise tensor_adds; mask apply is a fused
scalar_tensor_tensor: out = (sum >= 0) * x, written in-place, one op per
block-phase (i,j). DMAs are 1.6 MB, 128-partition, on the HWDGE (nc.sync).
"""
import sys

if "/opt/trn_rl_repo" not in sys.path:
    sys.path.insert(0, "/opt/trn_rl_repo")

import numpy as np
from contextlib import ExitStack

import concourse.tile as tile
from concourse import bacc, mybir
from concourse.bass_utils import run_bass_kernel_spmd

N_FULL, C, H, W = 32, 128, 112, 112
N_CORES = 8
N_PER_CORE = N_FULL // N_CORES  # 4
CHUNK_H = 28

_compiled = None


def _build():
    N = N_PER_CORE
    dt = mybir.dt.float32
    nc = bacc.Bacc("TRN2", target_bir_lowering=False, debug=False)
    x = nc.dram_tensor("x", [N, C, H, W], dt, kind="ExternalInput").ap()
    y = nc.dram_tensor("y", [N, C, H, W], dt, kind="ExternalOutput").ap()

    n_chunks = H // CHUNK_H
    F = CHUNK_H * W
    ge, mul = mybir.AluOpType.is_ge, mybir.AluOpType.mult

    with tile.TileContext(nc) as tc, ExitStack() as ctx:
        p1 = ctx.enter_context(tc.tile_pool(name="g1", bufs=3))
        p2 = ctx.enter_context(tc.tile_pool(name="g2", bufs=3))
        p3 = ctx.enter_context(tc.tile_pool(name="g3", bufs=3))
        tp = ctx.enter_context(tc.tile_pool(name="tmp", bufs=2))

        for ci in range(n_chunks):
            h0 = ci * CHUNK_H
            hs = slice(h0, h0 + CHUNK_H)

            # ---- G1: relu, channels [0,64), two (64ch x 2img) tiles ----
            for half in range(2):
                ns = slice(2 * half, 2 * half + 2)
                x1 = p1.tile([128, F], dt, tag=f"g1{half}")
                nc.sync.dma_start(
                    out=x1[:],
                    in_=x[ns, 0:64, hs, :].rearrange("n c h w -> c n (h w)"),
                )
                nc.scalar.activation(
                    x1[:], x1[:], mybir.ActivationFunctionType.Relu
                )
                nc.sync.dma_start(
                    out=y[ns, 0:64, hs, :].rearrange("n c h w -> c n (h w)"),
                    in_=x1[:],
                )

            # ---- G2: 2x2 blocks, channels [64,96) ----
            x2 = p2.tile([128, F], dt)
            nc.sync.dma_start(
                out=x2[:],
                in_=x[:, 64:96, hs, :].rearrange("n c h w -> c n (h w)"),
            )
            x2v = x2[:].rearrange("p (h w) -> p h w", h=CHUNK_H)
            s1 = tp.tile([128, CHUNK_H * (W // 2)], dt, tag="s1")
            s1v = s1[:].rearrange("p (h w) -> p h w", h=CHUNK_H)
            nc.vector.tensor_add(s1v, x2v[:, :, 0::2], x2v[:, :, 1::2])
            s2 = tp.tile([128, (CHUNK_H // 2) * (W // 2)], dt, tag="s2")
            s2v = s2[:].rearrange("p (h w) -> p h w", h=CHUNK_H // 2)
            nc.vector.tensor_add(s2v, s1v[:, 0::2, :], s1v[:, 1::2, :])
            for i in range(2):
                for j in range(2):
                    sub = x2v[:, i::2, j::2]
                    nc.vector.scalar_tensor_tensor(sub, s2v, 0.0, sub, ge, mul)
            nc.sync.dma_start(
                out=y[:, 64:96, hs, :].rearrange("n c h w -> c n (h w)"),
                in_=x2[:],
            )

            # ---- G3: 4x4 blocks [96,120) + identity [120,128) ----
            x3 = p3.tile([128, F], dt)
            nc.sync.dma_start(
                out=x3[:],
                in_=x[:, 96:128, hs, :].rearrange("n c h w -> c n (h w)"),
            )
            x3v = x3[0:96].rearrange("p (h w) -> p h w", h=CHUNK_H)
            t1 = tp.tile([96, CHUNK_H * (W // 2)], dt, tag="t1")
            t1v = t1[:].rearrange("p (h w) -> p h w", h=CHUNK_H)
            nc.vector.tensor_add(t1v, x3v[:, :, 0::2], x3v[:, :, 1::2])
            t2 = tp.tile([96, CHUNK_H * (W // 4)], dt, tag="t2")
            t2v = t2[:].rearrange("p (h w) -> p h w", h=CHUNK_H)
            nc.vector.tensor_add(t2v, t1v[:, :, 0::2], t1v[:, :, 1::2])
            t3 = tp.tile([96, (CHUNK_H // 2) * (W // 4)], dt, tag="t3")
            t3v = t3[:].rearrange("p (h w) -> p h w", h=CHUNK_H // 2)
            nc.vector.tensor_add(t3v, t2v[:, 0::2, :], t2v[:, 1::2, :])
            t4 = tp.tile([96, (CHUNK_H // 4) * (W // 4)], dt, tag="t4")
            t4v = t4[:].rearrange("p (h w) -> p h w", h=CHUNK_H // 4)
            nc.vector.tensor_add(t4v, t3v[:, 0::2, :], t3v[:, 1::2, :])
            for i in range(4):
                for j in range(4):
                    sub = x3v[:, i::4, j::4]
                    nc.vector.scalar_tensor_tensor(sub, t4v, 0.0, sub, ge, mul)
            nc.sync.dma_start(
                out=y[:, 96:128, hs, :].rearrange("n c h w -> c n (h w)"),
                in_=x3[:],
            )

    nc.compile()
    return nc


def _get_compiled():
    global _compiled
    if _compiled is None:
        _compiled = _build()
    return _compiled


def kernel(activation: np.ndarray, _trace: bool = False):
    nc = _get_compiled()
    activation = np.ascontiguousarray(activation, dtype=np.float32)
    in_maps = [
        {"x": activation[i * N_PER_CORE : (i + 1) * N_PER_CORE]}
        for i in range(N_CORES)
    ]
    res = run_bass_kernel_spmd(nc, in_maps, core_ids=list(range(N_CORES)),
                               trace=_trace)
    out = np.concatenate([r["y"] for r in res.results], axis=0)
    if _trace:
        return out, res
    return out


# revision 2
# speedup vs baseline: 1.0492x; 1.0492x over previous
"""BlockReLU Trainium2 kernel.

Full input: activation [32, 128, 112, 112] f32. Channel groups:
  [0,64): 1x1 blocks (plain ReLU), [64,96): 2x2 blocks, [96,120): 4x4 blocks,
  [120,128): identity passthrough.
A block's mask is 1 where the block's spatial sum >= 0, else 0; the mask is
broadcast over the block and multiplies the input.

Strategy: pure data parallelism over batch N across 8 NeuronCores (4 images
per core). Per core, stream H in chunks of 28 rows. For each chunk, pack
(channel, image) pairs onto all 128 SBUF partitions so every engine op uses
every lane:
  G1 relu:  two tiles, p = c*2 + n_local      (64ch x 2img) -> ScalarE Relu
  G2 2x2:   one tile,  p = (c-64)*4 + n       (32ch x 4img) -> VectorE
  G3+G4:    one tile,  p = (c-96)*4 + n       (32ch x 4img) -> VectorE
            (partitions 0:96 are the 4x4-mask channels, 96:128 identity)
Block sums are strided pairwise tensor_adds; mask apply is a fused
scalar_tensor_tensor: out = (sum >= 0) * x, written in-place, one op per
block-phase (i,j). DMAs are 1.6 MB, 128-partition, on the HWDGE (nc.sync).
"""
import sys

if "/opt/trn_rl_repo" not in sys.path:
    sys.path.insert(0, "/opt/trn_rl_repo")

import numpy as np
from contextlib import ExitStack

import concourse.tile as tile
from concourse import bacc, mybir
from concourse.bass_utils import run_bass_kernel_spmd

N_FULL, C, H, W = 32, 128, 112, 112
N_CORES = 8
N_PER_CORE = N_FULL // N_CORES  # 4
CHUNK_H = 28

_compiled = None


def _build():
    N = N_PER_CORE
    dt = mybir.dt.float32
    nc = bacc.Bacc("TRN2", target_bir_lowering=False, debug=False)
    x = nc.dram_tensor("x", [N, C, H, W], dt, kind="ExternalInput").ap()
    y = nc.dram_tensor("y", [N, C, H, W], dt, kind="ExternalOutput").ap()

    n_chunks = H // CHUNK_H
    F = CHUNK_H * W
    ge, mul = mybir.AluOpType.is_ge, mybir.AluOpType.mult

    with tile.TileContext(nc) as tc, ExitStack() as ctx:
        p1 = ctx.enter_context(tc.tile_pool(name="g1", bufs=3))
        p2 = ctx.enter_context(tc.tile_pool(name="g2", bufs=3))
        p3 = ctx.enter_context(tc.tile_pool(name="g3", bufs=3))
        tp = ctx.enter_context(tc.tile_pool(name="tmp", bufs=2))

        for ci in range(n_chunks):
            h0 = ci * CHUNK_H
            hs = slice(h0, h0 + CHUNK_H)

            # ---- G1: relu, channels [0,64), two (64ch x 2img) tiles ----
            for half in range(2):
                ns = slice(2 * half, 2 * half + 2)
                x1 = p1.tile([128, F], dt, tag=f"g1{half}")
                nc.sync.dma_start(
                    out=x1[:],
                    in_=x[ns, 0:64, hs, :].rearrange("n c h w -> c n (h w)"),
                )
                nc.scalar.activation(
                    x1[:], x1[:], mybir.ActivationFunctionType.Relu
                )
                nc.scalar.dma_start(
                    out=y[ns, 0:64, hs, :].rearrange("n c h w -> c n (h w)"),
                    in_=x1[:],
                )

            # ---- G2: 2x2 blocks, channels [64,96) ----
            x2 = p2.tile([128, F], dt)
            nc.sync.dma_start(
                out=x2[:],
                in_=x[:, 64:96, hs, :].rearrange("n c h w -> c n (h w)"),
            )
            x2v = x2[:].rearrange("p (h w) -> p h w", h=CHUNK_H)
            s1 = tp.tile([128, CHUNK_H * (W // 2)], dt, tag="s1")
            s1v = s1[:].rearrange("p (h w) -> p h w", h=CHUNK_H)
            nc.vector.tensor_add(s1v, x2v[:, :, 0::2], x2v[:, :, 1::2])
            s2 = tp.tile([128, (CHUNK_H // 2) * (W // 2)], dt, tag="s2")
            s2v = s2[:].rearrange("p (h w) -> p h w", h=CHUNK_H // 2)
            nc.vector.tensor_add(s2v, s1v[:, 0::2, :], s1v[:, 1::2, :])
            for i in range(2):
                for j in range(2):
                    sub = x2v[:, i::2, j::2]
                    nc.vector.scalar_tensor_tensor(sub, s2v, 0.0, sub, ge, mul)
            nc.scalar.dma_start(
                out=y[:, 64:96, hs, :].rearrange("n c h w -> c n (h w)"),
                in_=x2[:],
            )

            # ---- G3: 4x4 blocks [96,120) + identity [120,128) ----
            x3 = p3.tile([128, F], dt)
            nc.sync.dma_start(
                out=x3[:],
                in_=x[:, 96:128, hs, :].rearrange("n c h w -> c n (h w)"),
            )
            x3v = x3[0:96].rearrange("p (h w) -> p h w", h=CHUNK_H)
            t1 = tp.tile([96, CHUNK_H * (W // 2)], dt, tag="t1")
            t1v = t1[:].rearrange("p (h w) -> p h w", h=CHUNK_H)
            nc.vector.tensor_add(t1v, x3v[:, :, 0::2], x3v[:, :, 1::2])
            t2 = tp.tile([96, CHUNK_H * (W // 4)], dt, tag="t2")
            t2v = t2[:].rearrange("p (h w) -> p h w", h=CHUNK_H)
            nc.vector.tensor_add(t2v, t1v[:, :, 0::2], t1v[:, :, 1::2])
            t3 = tp.tile([96, (CHUNK_H // 2) * (W // 4)], dt, tag="t3")
            t3v = t3[:].rearrange("p (h w) -> p h w", h=CHUNK_H // 2)
            nc.vector.tensor_add(t3v, t2v[:, 0::2, :], t2v[:, 1::2, :])
            t4 = tp.tile([96, (CHUNK_H // 4) * (W // 4)], dt, tag="t4")
            t4v = t4[:].rearrange("p (h w) -> p h w", h=CHUNK_H // 4)
            nc.vector.tensor_add(t4v, t3v[:, 0::2, :], t3v[:, 1::2, :])
            for i in range(4):
                for j in range(4):
                    sub = x3v[:, i::4, j::4]
                    nc.vector.scalar_tensor_tensor(sub, t4v, 0.0, sub, ge, mul)
            nc.scalar.dma_start(
                out=y[:, 96:128, hs, :].rearrange("n c h w -> c n (h w)"),
                in_=x3[:],
            )

    nc.compile()
    return nc


def _get_compiled():
    global _compiled
    if _compiled is None:
        _compiled = _build()
    return _compiled


def kernel(activation: np.ndarray, _trace: bool = False):
    nc = _get_compiled()
    activation = np.ascontiguousarray(activation, dtype=np.float32)
    in_maps = [
        {"x": activation[i * N_PER_CORE : (i + 1) * N_PER_CORE]}
        for i in range(N_CORES)
    ]
    res = run_bass_kernel_spmd(nc, in_maps, core_ids=list(range(N_CORES)),
                               trace=_trace)
    out = np.concatenate([r["y"] for r in res.results], axis=0)
    if _trace:
        return out, res
    return out
